# revision 20
# baseline (speedup 1.0000x reference)
"""BlockGRUCell Trainium2 kernel.

Computation (per reference):
  hx = concat([h, x], -1)                       # (B, 2048)
  gate[b, 192g+o] = sum_i hx[b, 128g+i] * W[g, o, i]   # block-diagonal matmul
  r, c, u = split(gate + bias, 3)               # bias == 0 from setup_inputs
  h_new = sigmoid(u) * tanh(sigmoid(r) * c) + (1 - sigmoid(u)) * h

Sharding: data-parallel over batch across 8 NeuronCores (2048 rows each),
weights replicated.

The TensorE matmul contracts over the partition dim, so the stationary
operand must be hx^T per 128-feature block. The host pre-packs hx into
per-tile transposed bf16 panels (doing this on device costs a PE transpose
plus a PSUM->SBUF cast that saturates VectorE/ScalarE):
  hxt[t, p, 128g+b] = hx[128t+b, 128g+p]

Per core, per 128-row tile:
  - DMA: hxt tile (bf16 transposed panel, 512K), h tile (fp32, 512K)
  - 20 block matmuls (bf16, fp32 accum) into three [128, 1024] PSUM panels
    (= r/c/u exactly; matmuls split at PSUM bank crossings); pool bufs=4
    so the next tile's r-matmuls start as soon as one panel frees
  - ScalarE: sigmoid(r), tanh(reset*c), sigmoid(u)
  - VectorE: rc from PSUM and the blend h + upd*(cand - h); fp32
    tensor_tensor is 1x everywhere and GpSimd would steal DVE's second
    read port, so everything elementwise stays on VectorE
"""

import numpy as np
import ml_dtypes

import concourse.bass as bass
import concourse.bacc as bacc
import concourse.tile as tile
import concourse.mybir as mybir
from concourse.bass_utils import run_bass_kernel_spmd

N_CORES = 8
BATCH = 16384
BS = BATCH // N_CORES            # rows per core
P = 128
NT = BS // P                     # 128-row tiles per core
HID = 1024
G = 16                           # feature blocks
IN_PER = 128
OUT_PER = 192
GATE = 3 * HID                   # 3072
PSUM_BANK_F32 = 512

def _make_segs():
    bounds = sorted({0, GATE} |
                    {k * OUT_PER for k in range(G + 1)} |
                    {k * PSUM_BANK_F32 for k in range(GATE // PSUM_BANK_F32 + 1)})
    segs = [(a, b) for a, b in zip(bounds, bounds[1:])]
    order = {0: 0, 2: 1, 1: 2}          # R, U, C emission order
    return sorted(segs, key=lambda s: (order[s[0] // HID], s[0]))

_SEGS = _make_segs()
_SEGS0 = sorted(_SEGS, key=lambda sg: ({0: 0, 2: 1, 1: 2}[sg[0] // HID],
                                       sg[0]))

F32 = mybir.dt.float32
BF16 = mybir.dt.bfloat16
F16 = mybir.dt.float16
AFT = mybir.ActivationFunctionType


def _body(tc, nc, hxt_d, id_d, wt_d, out_d):
    with (
        tc.tile_pool(name="consts", bufs=1) as consts,
        tc.tile_pool(name="io", bufs=6) as io,
        tc.tile_pool(name="panels", bufs=4) as panels,
        tc.tile_pool(name="gatep", bufs=3, space="PSUM") as gatep,
        tc.tile_pool(name="htp", bufs=2, space="PSUM") as htp,
    ):
        # warm the sigmoid/tanh ACT table during the initial DMAs (the
        # ~2.7us ACT_TABLE_LOAD otherwise lands on tile 0's critical path)
        warm = consts.tile([P, 1], F32)
        nc.vector.memset(warm, 0.0)
        nc.scalar.activation(warm, warm, AFT.Sigmoid)

        ident = consts.tile([P, P], BF16)
        nc.sync.dma_start(out=ident, in_=id_d[:, :])

        # split the weight load per gate so tile 0's r matmuls start sooner
        wt_s = consts.tile([P, G * OUT_PER], BF16)
        for k in range(3):
            nc.sync.dma_start(out=wt_s[:, k * HID:(k + 1) * HID],
                              in_=wt_d[:, k * HID:(k + 1) * HID])

        # Software pipeline at pair granularity (two 128-row tiles).
        # Engines execute their instruction streams in order, so the
        # emission order IS the schedule:
        #   iter 2p:   matmuls_2p; sig(r); rc [DVE]; sig(u)
        #   iter 2p+1: matmuls_2p+1; sig(r); rc; sig(u);
        #              blend of pair p-1 [DVE, 2048-col ops];
        #              tanh of pair p [ACT, one 2048-col op]
        # tanh/blend run one-to-two tiles behind the gates so neither ACT
        # nor DVE ever waits mid-stream; 2048-col instructions amortize the
        # fixed per-instruction overheads (~150ns ACT, ~60-125ns DVE).
        st = {}

        def stage_front(t):
            hxt = st[("hxt", t)]
            gR = gatep.tile([P, HID], F32, tag="gate")
            gC = gatep.tile([P, HID], F32, tag="gate")
            gU = gatep.tile([P, HID], F32, tag="gate")
            gs = (gR, gC, gU)
            # segments split at block (192) and PSUM bank (512) boundaries,
            # emitted R, C, U so rc (the DVE stream head) unblocks early;
            # tile 0 goes R, U, C so ACT starts earliest and never waits
            for c0, c1 in (_SEGS if t > 0 else _SEGS0):
                g = c0 // OUT_PER
                gate = gs[c0 // HID]
                nc.tensor.matmul(gate[:, c0 % HID:(c0 % HID) + c1 - c0],
                                 hxt[:, g * P:(g + 1) * P], wt_s[:, c0:c1],
                                 start=True, stop=True)
            if t % 2 == 0:
                rc2_new = panels.tile([P, 2 * HID], F32, tag="rc2", bufs=2)
                upd2_new = panels.tile([P, 2 * HID], BF16, tag="upd2", bufs=2)
                st["rc2"], st["upd2"] = rc2_new, upd2_new
            rc2, upd2 = st["rc2"], st["upd2"]
            o = (t % 2) * HID
            # reset/rc outputs stay fp32: a bf16 output on the PSUM-reading
            # tensor_tensor costs ~2.3x on DVE
            reset = panels.tile([P, HID], F32, tag="reset")
            # tile 0 runs in halves so ACT starts right after the first
            # R-block matmuls instead of waiting for all of them
            if t > 0:
                nc.scalar.activation(reset, gR, AFT.Sigmoid)
                nc.vector.tensor_tensor(rc2[:, o:o + HID], gC,
                                        reset, mybir.AluOpType.mult)
                nc.scalar.activation(upd2[:, o:o + HID], gU, AFT.Sigmoid)
            else:
                # tile 0 ramp: sigmoid(r) in quarters right behind the R
                # matmuls, sigmoid(u) in halves, rc once C lands
                Q = HID // 4
                for k in range(4):
                    nc.scalar.activation(reset[:, k * Q:(k + 1) * Q],
                                         gR[:, k * Q:(k + 1) * Q],
                                         AFT.Sigmoid)
                for a, b in ((0, HID // 2), (HID // 2, HID)):
                    nc.scalar.activation(upd2[:, o + a:o + b], gU[:, a:b],
                                         AFT.Sigmoid)
                nc.vector.tensor_tensor(rc2[:, o:o + HID], gC,
                                        reset, mybir.AluOpType.mult)
            if t % 2 == 1:
                st[("rc2", t // 2)] = st.pop("rc2")
                st[("upd2", t // 2)] = st.pop("upd2")

        def stage_mid(p, last=False):
            rc2 = st.pop(("rc2", p))
            cand2 = panels.tile([P, 2 * HID], BF16, tag="cand2", bufs=2)
            if last:
                # finer drain: the final tanh lands in quarters
                Q = HID // 2
                for k in range(4):
                    nc.scalar.activation(cand2[:, k * Q:(k + 1) * Q],
                                         rc2[:, k * Q:(k + 1) * Q], AFT.Tanh)
            else:
                nc.scalar.activation(cand2, rc2, AFT.Tanh)
            st[("cand2", p)] = cand2
            # h for the blend: PE-transpose hxt's h-half back to row-major
            # PSUM bf16 (saves the 4MB h DMA; bf16 keeps DVE 2x even from
            # PSUM)
            for k in range(2):
                hxt = st.pop(("hxt", 2 * p + k))
                hT = htp.tile([P, HID], BF16, tag="hT")
                for g in range(8):
                    nc.tensor.transpose(hT[:, g * P:(g + 1) * P],
                                        hxt[:, g * P:(g + 1) * P], ident)
                st[("hT", 2 * p + k)] = hT

        def stage_back(p, quarters=False):
            cand2 = st.pop(("cand2", p))
            upd2 = st.pop(("upd2", p))
            hTs = (st.pop(("hT", 2 * p)), st.pop(("hT", 2 * p + 1)))
            dd2 = panels.tile([P, 2 * HID], BF16, tag="dd2", bufs=2)
            ee2 = panels.tile([P, 2 * HID], BF16, tag="ee2", bufs=2)
            out2 = io.tile([P, 2 * HID], BF16, tag="out", bufs=3)
            Q = HID // 2
            hsplits = [(0, HID), (HID, 2 * HID)] if not quarters else \
                      [(k * Q, (k + 1) * Q) for k in range(4)]
            for a, b in hsplits:
                hT = hTs[a // HID]
                nc.vector.tensor_sub(dd2[:, a:b], cand2[:, a:b],
                                     hT[:, a % HID:(b - 1) % HID + 1])
            if not quarters:
                # the ee multiply has all-SBUF operands: run it pair-wide
                nc.vector.tensor_mul(ee2, upd2, dd2)
            for a, b in hsplits:
                hT = hTs[a // HID]
                ha, hb = a % HID, (b - 1) % HID + 1
                if quarters:
                    nc.vector.tensor_mul(ee2[:, a:b], upd2[:, a:b],
                                         dd2[:, a:b])
                nc.vector.tensor_add(out2[:, a:b], hT[:, ha:hb], ee2[:, a:b])
                nc.sync.dma_start(out=out_d[2 * p + (a >= HID)][:, ha:hb],
                                  in_=out2[:, a:b])

        NP = NT // 2
        for t in range(NT):
            if t == 0:
                hxt = hxt0
            else:
                hxt = io.tile([P, G * P], BF16, tag="hxt")
                nc.sync.dma_start(out=hxt, in_=hxt_d[t])
            st[("hxt", t)] = hxt
            stage_front(t)
            if t % 2 == 1:
                p = t // 2
                if p >= 1:
                    stage_back(p - 1)
                stage_mid(p, last=(p == NP - 1))
        stage_back(NP - 1, quarters=True)

_NC_CACHE = {}


def _build_nc():
    if "nc" in _NC_CACHE:
        return _NC_CACHE["nc"]
    nc = bacc.Bacc()
    hxt_d = nc.dram_tensor("hxt", [NT, P, G * P], BF16, kind="ExternalInput")
    id_d = nc.dram_tensor("ident", [P, P], BF16, kind="ExternalInput")
    wt_d = nc.dram_tensor("wt", [P, G * OUT_PER], BF16, kind="ExternalInput")
    out_d = nc.dram_tensor("out", [NT, P, HID], BF16,
                           kind="ExternalOutput")
    with tile.TileContext(nc) as tc:
        _body(tc, nc, hxt_d, id_d, wt_d, out_d)
    nc.compile()
    _NC_CACHE["nc"] = nc
    return nc


def _np_reference(x, h, weight, bias):
    hx = np.concatenate([h, x], axis=-1)
    xg = hx.reshape(x.shape[0], G, IN_PER)
    gate = np.einsum("bgi,goi->bgo", xg, weight).reshape(x.shape[0], GATE)
    gate = gate + bias
    r, c, u = np.split(gate, 3, axis=-1)
    reset = 1.0 / (1.0 + np.exp(-r))
    cand = np.tanh(reset * c)
    upd = 1.0 / (1.0 + np.exp(-u))
    return (upd * cand + (1.0 - upd) * h).astype(np.float32)


def _pack_hxt(hs, xs):
    """-> [NT, 128, 2048] bf16 with hxt[t, p, 128g+b] = hx[128t+b, 128g+p],
    where hx = concat([h, x], -1) per-row (blocks 0-7 = h, 8-15 = x)."""
    def tp(a):                      # [BS, 1024] -> [NT, 128, 8, 128]
        return a.reshape(NT, P, 8, P).transpose(0, 3, 2, 1)   # [t, p, g, b]
    arr = np.concatenate([tp(hs), tp(xs)], axis=2)            # [t, p, 16, b]
    return np.ascontiguousarray(arr.reshape(NT, P, G * P)).astype(
        ml_dtypes.bfloat16)


def _pack_pairs(a):
    """[BS, 1024] -> [NT//2, 128, 2048]: [q, p, 1024*s+f] = a[256q+128s+p, f]."""
    return np.ascontiguousarray(
        a.reshape(NT // 2, 2, P, HID).transpose(0, 2, 1, 3)
        .reshape(NT // 2, P, 2 * HID))


def _run(x, h, weight, bias, trace=False, tmpdir=None):
    # wt[p, 192g+o] = W[g, o, p] — the exact SBUF layout, one contiguous DMA
    wt = np.ascontiguousarray(
        weight.transpose(2, 0, 1).reshape(P, G * OUT_PER)).astype(
        ml_dtypes.bfloat16)
    nc = _build_nc()
    ident = np.eye(P, dtype=np.float32).astype(ml_dtypes.bfloat16)
    in_maps = []
    for c in range(N_CORES):
        sl = slice(c * BS, (c + 1) * BS)
        xs, hs = x[sl], h[sl]
        in_maps.append({
            "hxt": _pack_hxt(hs, xs),
            "ident": ident,
            "wt": wt,
        })
    res = run_bass_kernel_spmd(nc, in_maps, core_ids=list(range(N_CORES)),
                               trace=trace, tmpdir=tmpdir)
    out = np.concatenate(
        [m["out"].astype(np.float32).reshape(BS, HID) for m in res.results],
        axis=0)
    return out, res


def kernel(x, h, weight, bias):
    x = np.asarray(x, dtype=np.float32)
    h = np.asarray(h, dtype=np.float32)
    weight = np.asarray(weight, dtype=np.float32)
    bias = np.asarray(bias, dtype=np.float32)
    if np.any(bias != 0.0):
        # setup_inputs() always passes zero bias; keep a correct fallback.
        return _np_reference(x, h, weight, bias)
    out, _ = _run(x, h, weight, bias)
    return out



# revision 22
# speedup vs baseline: 1.0173x; 1.0173x over previous
"""BlockGRUCell Trainium2 kernel.

Computation (per reference):
  hx = concat([h, x], -1)                       # (B, 2048)
  gate[b, 192g+o] = sum_i hx[b, 128g+i] * W[g, o, i]   # block-diagonal matmul
  r, c, u = split(gate + bias, 3)               # bias == 0 from setup_inputs
  h_new = sigmoid(u) * tanh(sigmoid(r) * c) + (1 - sigmoid(u)) * h

Sharding: data-parallel over batch across 8 NeuronCores (2048 rows each),
weights replicated.

The TensorE matmul contracts over the partition dim, so the stationary
operand must be hx^T per 128-feature block. The host pre-packs hx into
per-tile transposed bf16 panels (doing this on device costs a PE transpose
plus a PSUM->SBUF cast that saturates VectorE/ScalarE):
  hxt[t, p, 128g+b] = hx[128t+b, 128g+p]

Per core, per 128-row tile:
  - DMA: hxt tile (bf16 transposed panel, 512K), h tile (fp32, 512K)
  - 20 block matmuls (bf16, fp32 accum) into three [128, 1024] PSUM panels
    (= r/c/u exactly; matmuls split at PSUM bank crossings); pool bufs=4
    so the next tile's r-matmuls start as soon as one panel frees
  - ScalarE: sigmoid(r), tanh(reset*c), sigmoid(u)
  - VectorE: rc from PSUM and the blend h + upd*(cand - h); fp32
    tensor_tensor is 1x everywhere and GpSimd would steal DVE's second
    read port, so everything elementwise stays on VectorE
"""

import numpy as np
import ml_dtypes

import concourse.bass as bass
import concourse.bacc as bacc
import concourse.tile as tile
import concourse.mybir as mybir
from concourse.bass_utils import run_bass_kernel_spmd

N_CORES = 8
BATCH = 16384
BS = BATCH // N_CORES            # rows per core
P = 128
NT = BS // P                     # 128-row tiles per core
HID = 1024
G = 16                           # feature blocks
IN_PER = 128
OUT_PER = 192
GATE = 3 * HID                   # 3072
PSUM_BANK_F32 = 512

def _make_segs():
    bounds = sorted({0, GATE} |
                    {k * OUT_PER for k in range(G + 1)} |
                    {k * PSUM_BANK_F32 for k in range(GATE // PSUM_BANK_F32 + 1)})
    segs = [(a, b) for a, b in zip(bounds, bounds[1:])]
    order = {0: 0, 2: 1, 1: 2}          # R, U, C emission order
    return sorted(segs, key=lambda s: (order[s[0] // HID], s[0]))

_SEGS = _make_segs()

F32 = mybir.dt.float32
BF16 = mybir.dt.bfloat16
F16 = mybir.dt.float16
AFT = mybir.ActivationFunctionType


def _body(tc, nc, hxt_d, id_d, wt_d, out_d):
    with (
        tc.tile_pool(name="consts", bufs=1) as consts,
        tc.tile_pool(name="io", bufs=6) as io,
        tc.tile_pool(name="panels", bufs=4) as panels,
        tc.tile_pool(name="gatep", bufs=3, space="PSUM") as gatep,
        tc.tile_pool(name="htp", bufs=2, space="PSUM") as htp,
    ):
        # warm the sigmoid/tanh ACT table during the initial DMAs (the
        # ~2.7us ACT_TABLE_LOAD otherwise lands on tile 0's critical path)
        warm = consts.tile([P, 1], F32)
        nc.vector.memset(warm, 0.0)
        nc.scalar.activation(warm, warm, AFT.Sigmoid)

        ident = consts.tile([P, P], BF16)
        nc.sync.dma_start(out=ident, in_=id_d[:, :])

        # split the weight load per gate so tile 0's r matmuls start sooner
        wt_s = consts.tile([P, G * OUT_PER], BF16)
        for k in range(3):
            nc.sync.dma_start(out=wt_s[:, k * HID:(k + 1) * HID],
                              in_=wt_d[:, k * HID:(k + 1) * HID])

        # Software pipeline at pair granularity (two 128-row tiles).
        # Engines execute their instruction streams in order, so the
        # emission order IS the schedule:
        #   iter 2p:   matmuls_2p; sig(r); rc [DVE]; sig(u)
        #   iter 2p+1: matmuls_2p+1; sig(r); rc; sig(u);
        #              blend of pair p-1 [DVE, 2048-col ops];
        #              tanh of pair p [ACT, one 2048-col op]
        # tanh/blend run one-to-two tiles behind the gates so neither ACT
        # nor DVE ever waits mid-stream; 2048-col instructions amortize the
        # fixed per-instruction overheads (~150ns ACT, ~60-125ns DVE).
        st = {}

        def stage_front(t):
            hxt = st[("hxt", t)]
            gR = gatep.tile([P, HID], F32, tag="gate")
            gC = gatep.tile([P, HID], F32, tag="gate")
            gU = gatep.tile([P, HID], F32, tag="gate")
            gs = (gR, gC, gU)
            # segments split at block (192) and PSUM bank (512) boundaries,
            # emitted R, C, U so rc (the DVE stream head) unblocks early
            for c0, c1 in _SEGS:
                g = c0 // OUT_PER
                gate = gs[c0 // HID]
                nc.tensor.matmul(gate[:, c0 % HID:(c0 % HID) + c1 - c0],
                                 hxt[:, g * P:(g + 1) * P], wt_s[:, c0:c1],
                                 start=True, stop=True)
            if t % 2 == 0:
                rc2_new = panels.tile([P, 2 * HID], F32, tag="rc2", bufs=2)
                upd2_new = panels.tile([P, 2 * HID], BF16, tag="upd2", bufs=2)
                st["rc2"], st["upd2"] = rc2_new, upd2_new
            rc2, upd2 = st["rc2"], st["upd2"]
            o = (t % 2) * HID
            # reset/rc outputs stay fp32: a bf16 output on the PSUM-reading
            # tensor_tensor costs ~2.3x on DVE
            reset = panels.tile([P, HID], F32, tag="reset")
            # tile 0 runs in halves so ACT starts right after the first
            # R-block matmuls instead of waiting for all of them
            sp = [(0, HID)] if t > 0 else \
                 [(k * HID // 4, (k + 1) * HID // 4) for k in range(4)]
            for a, b in sp:
                nc.scalar.activation(reset[:, a:b], gR[:, a:b], AFT.Sigmoid)
                nc.vector.tensor_tensor(rc2[:, o + a:o + b], gC[:, a:b],
                                        reset[:, a:b], mybir.AluOpType.mult)
                nc.scalar.activation(upd2[:, o + a:o + b], gU[:, a:b],
                                     AFT.Sigmoid)
            if t % 2 == 1:
                st[("rc2", t // 2)] = st.pop("rc2")
                st[("upd2", t // 2)] = st.pop("upd2")

        def stage_mid(p):
            rc2 = st.pop(("rc2", p))
            cand2 = panels.tile([P, 2 * HID], BF16, tag="cand2", bufs=2)
            nc.scalar.activation(cand2, rc2, AFT.Tanh)
            st[("cand2", p)] = cand2
            # h for the blend: PE-transpose hxt's h-half back to row-major
            # PSUM bf16 (saves the 4MB h DMA; bf16 keeps DVE 2x even from
            # PSUM)
            for k in range(2):
                hxt = st.pop(("hxt", 2 * p + k))
                hT = htp.tile([P, HID], BF16, tag="hT")
                for g in range(8):
                    nc.tensor.transpose(hT[:, g * P:(g + 1) * P],
                                        hxt[:, g * P:(g + 1) * P], ident)
                st[("hT", 2 * p + k)] = hT

        def stage_back(p, quarters=False):
            cand2 = st.pop(("cand2", p))
            upd2 = st.pop(("upd2", p))
            hTs = (st.pop(("hT", 2 * p)), st.pop(("hT", 2 * p + 1)))
            dd2 = panels.tile([P, 2 * HID], BF16, tag="dd2", bufs=2)
            ee2 = panels.tile([P, 2 * HID], BF16, tag="ee2", bufs=2)
            out2 = io.tile([P, 2 * HID], BF16, tag="out", bufs=3)
            Q = HID // 2
            hsplits = [(0, HID), (HID, 2 * HID)] if not quarters else \
                      [(k * Q, (k + 1) * Q) for k in range(4)]
            for a, b in hsplits:
                hT = hTs[a // HID]
                nc.vector.tensor_sub(dd2[:, a:b], cand2[:, a:b],
                                     hT[:, a % HID:(b - 1) % HID + 1])
            if not quarters:
                # the ee multiply has all-SBUF operands: run it pair-wide
                nc.vector.tensor_mul(ee2, upd2, dd2)
            for a, b in hsplits:
                hT = hTs[a // HID]
                ha, hb = a % HID, (b - 1) % HID + 1
                if quarters:
                    nc.vector.tensor_mul(ee2[:, a:b], upd2[:, a:b],
                                         dd2[:, a:b])
                nc.vector.tensor_add(out2[:, a:b], hT[:, ha:hb], ee2[:, a:b])
                nc.sync.dma_start(out=out_d[2 * p + (a >= HID)][:, ha:hb],
                                  in_=out2[:, a:b])

        NP = NT // 2
        for t in range(NT):
            if t == 0:
                hxt = hxt0
            else:
                hxt = io.tile([P, G * P], BF16, tag="hxt")
                nc.sync.dma_start(out=hxt, in_=hxt_d[t])
            st[("hxt", t)] = hxt
            if t == NT - 1:
                break                      # tile NT-1 is handled by the drain
            stage_front(t)
            if t % 2 == 1:
                p = t // 2
                if p >= 1:
                    stage_back(p - 1)
                if p < NP - 1:
                    stage_mid(p)
        # ---- tapered drain for the last pair (tiles NT-2, NT-1): the
        # blend of tile NT-2 overlaps tile NT-1's sigmoids, and only the
        # final half-column blend trails the last tanh ----
        rc2, upd2 = st["rc2"], st["upd2"]
        cand2 = panels.tile([P, 2 * HID], BF16, tag="cand2", bufs=2)
        HH = HID // 2
        # tanh for tile NT-2 in halves (rc for it was emitted by stage_front)
        for a in (0, HH):
            nc.scalar.activation(cand2[:, a:a + HH], rc2[:, a:a + HH],
                                 AFT.Tanh)
        hxtA = st.pop(("hxt", NT - 2))
        hTA = htp.tile([P, HID], BF16, tag="hT")
        for g in range(8):
            nc.tensor.transpose(hTA[:, g * P:(g + 1) * P],
                                hxtA[:, g * P:(g + 1) * P], ident)
        stage_front(NT - 1)
        st.pop(("rc2", NP - 1)); st.pop(("upd2", NP - 1))
        stage_back(NP - 2)
        outA = io.tile([P, HID], BF16, tag="out", bufs=3)
        ddA = panels.tile([P, HID], BF16, tag="dd2", bufs=2)
        eeA = panels.tile([P, HID], BF16, tag="ee2", bufs=2)
        for a in (0, HH):
            nc.vector.tensor_sub(ddA[:, a:a + HH], cand2[:, a:a + HH],
                                 hTA[:, a:a + HH])
            nc.vector.tensor_mul(eeA[:, a:a + HH], upd2[:, a:a + HH],
                                 ddA[:, a:a + HH])
            nc.vector.tensor_add(outA[:, a:a + HH], hTA[:, a:a + HH],
                                 eeA[:, a:a + HH])
            nc.sync.dma_start(out=out_d[NT - 2][:, a:a + HH],
                              in_=outA[:, a:a + HH])
        hxtB = st.pop(("hxt", NT - 1))
        hTB = htp.tile([P, HID], BF16, tag="hT")
        for g in range(8):
            nc.tensor.transpose(hTB[:, g * P:(g + 1) * P],
                                hxtB[:, g * P:(g + 1) * P], ident)
        outB = io.tile([P, HID], BF16, tag="out", bufs=3)
        ddB = panels.tile([P, HID], BF16, tag="dd3", bufs=1)
        eeB = panels.tile([P, HID], BF16, tag="ee3", bufs=1)
        for a in (0, HH):
            nc.scalar.activation(cand2[:, HID + a:HID + a + HH],
                                 rc2[:, HID + a:HID + a + HH], AFT.Tanh)
            nc.vector.tensor_sub(ddB[:, a:a + HH],
                                 cand2[:, HID + a:HID + a + HH],
                                 hTB[:, a:a + HH])
            nc.vector.tensor_mul(eeB[:, a:a + HH],
                                 upd2[:, HID + a:HID + a + HH],
                                 ddB[:, a:a + HH])
            nc.vector.tensor_add(outB[:, a:a + HH], hTB[:, a:a + HH],
                                 eeB[:, a:a + HH])
            nc.sync.dma_start(out=out_d[NT - 1][:, a:a + HH],
                              in_=outB[:, a:a + HH])

_NC_CACHE = {}


def _build_nc():
    if "nc" in _NC_CACHE:
        return _NC_CACHE["nc"]
    nc = bacc.Bacc()
    hxt_d = nc.dram_tensor("hxt", [NT, P, G * P], BF16, kind="ExternalInput")
    id_d = nc.dram_tensor("ident", [P, P], BF16, kind="ExternalInput")
    wt_d = nc.dram_tensor("wt", [P, G * OUT_PER], BF16, kind="ExternalInput")
    out_d = nc.dram_tensor("out", [NT, P, HID], BF16,
                           kind="ExternalOutput")
    with tile.TileContext(nc) as tc:
        _body(tc, nc, hxt_d, id_d, wt_d, out_d)
    nc.compile()
    _NC_CACHE["nc"] = nc
    return nc


def _np_reference(x, h, weight, bias):
    hx = np.concatenate([h, x], axis=-1)
    xg = hx.reshape(x.shape[0], G, IN_PER)
    gate = np.einsum("bgi,goi->bgo", xg, weight).reshape(x.shape[0], GATE)
    gate = gate + bias
    r, c, u = np.split(gate, 3, axis=-1)
    reset = 1.0 / (1.0 + np.exp(-r))
    cand = np.tanh(reset * c)
    upd = 1.0 / (1.0 + np.exp(-u))
    return (upd * cand + (1.0 - upd) * h).astype(np.float32)


def _pack_hxt(hs, xs):
    """-> [NT, 128, 2048] bf16 with hxt[t, p, 128g+b] = hx[128t+b, 128g+p],
    where hx = concat([h, x], -1) per-row (blocks 0-7 = h, 8-15 = x)."""
    def tp(a):                      # [BS, 1024] -> [NT, 128, 8, 128]
        return a.reshape(NT, P, 8, P).transpose(0, 3, 2, 1)   # [t, p, g, b]
    arr = np.concatenate([tp(hs), tp(xs)], axis=2)            # [t, p, 16, b]
    return np.ascontiguousarray(arr.reshape(NT, P, G * P)).astype(
        ml_dtypes.bfloat16)


def _pack_pairs(a):
    """[BS, 1024] -> [NT//2, 128, 2048]: [q, p, 1024*s+f] = a[256q+128s+p, f]."""
    return np.ascontiguousarray(
        a.reshape(NT // 2, 2, P, HID).transpose(0, 2, 1, 3)
        .reshape(NT // 2, P, 2 * HID))


def _run(x, h, weight, bias, trace=False, tmpdir=None):
    # wt[p, 192g+o] = W[g, o, p] — the exact SBUF layout, one contiguous DMA
    wt = np.ascontiguousarray(
        weight.transpose(2, 0, 1).reshape(P, G * OUT_PER)).astype(
        ml_dtypes.bfloat16)
    nc = _build_nc()
    ident = np.eye(P, dtype=np.float32).astype(ml_dtypes.bfloat16)
    in_maps = []
    for c in range(N_CORES):
        sl = slice(c * BS, (c + 1) * BS)
        xs, hs = x[sl], h[sl]
        in_maps.append({
            "hxt": _pack_hxt(hs, xs),
            "ident": ident,
            "wt": wt,
        })
    res = run_bass_kernel_spmd(nc, in_maps, core_ids=list(range(N_CORES)),
                               trace=trace, tmpdir=tmpdir)
    out = np.concatenate(
        [m["out"].astype(np.float32).reshape(BS, HID) for m in res.results],
        axis=0)
    return out, res


def kernel(x, h, weight, bias):
    x = np.asarray(x, dtype=np.float32)
    h = np.asarray(h, dtype=np.float32)
    weight = np.asarray(weight, dtype=np.float32)
    bias = np.asarray(bias, dtype=np.float32)
    if np.any(bias != 0.0):
        # setup_inputs() always passes zero bias; keep a correct fallback.
        return _np_reference(x, h, weight, bias)
    out, _ = _run(x, h, weight, bias)
    return out

